# revision 1
# baseline (speedup 1.0000x reference)
"""Trainium2 kernel for nn_Attention_33 (9-tile channel-attention, Restormer-style).

Strategy: the computation decomposes into 9 tiles x 4 batch = 36 fully
independent (tile, batch) work items (the attention is per-item; no
cross-item reduction).  We shard the 36 items across the 8 NeuronCores
(5 slots per core, 4 dummy slots) and run the per-item fused block on
each core; the host reassembles the 3x3 tile grid.
"""
import numpy as np

B, C, H, W = 4, 128, 384, 384
HEADS = 8
T = 9
HH, WW = H // 3, W // 3          # 128, 128
N_CORES = 8
SLOTS = 5                        # ceil(36/8)

_jit_cache = {}


def _get_runner():
    if "run" in _jit_cache:
        return _jit_cache["run"]

    import jax
    import jax.numpy as jnp
    from jax import lax

    def _item(x, ln_w, ln_b, qkv_w, qkv_b, dw_w, dw_b, proj_w, proj_b, temp, grw):
        # x: [C, HH, WW] one (tile, batch) item
        c, h, w = x.shape
        res = x
        mu = jnp.mean(x, axis=0, keepdims=True)
        var = jnp.mean((x - mu) ** 2, axis=0, keepdims=True)
        y = (x - mu) / jnp.sqrt(var + 1e-5) * ln_w[:, None, None] + ln_b[:, None, None]
        qkv = jnp.einsum('chw,oc->ohw', y, qkv_w) + qkv_b[:, None, None]
        # depthwise 3x3, padding 1, as 9 shifted multiply-adds (XLA-friendly)
        qp = jnp.pad(qkv, ((0, 0), (1, 1), (1, 1)))
        acc = dw_b[:, None, None]
        for dr in range(3):
            for dc in range(3):
                acc = acc + dw_w[:, 0, dr, dc, None, None] * \
                    lax.dynamic_slice(qp, (0, dr, dc), (3 * c, h, w))
        qkv = acc
        q, k, v = jnp.split(qkv, 3, axis=0)
        heads = lambda t_: t_.reshape(HEADS, c // HEADS, h * w)
        q, k, v = heads(q), heads(k), heads(v)
        q = q / jnp.maximum(jnp.linalg.norm(q, axis=-1, keepdims=True), 1e-12)
        k = k / jnp.maximum(jnp.linalg.norm(k, axis=-1, keepdims=True), 1e-12)
        attn = jnp.einsum('hcn,hdn->hcd', q, k) * temp[:, None, None]
        attn = jax.nn.softmax(attn, axis=-1)
        out = jnp.einsum('hcd,hdn->hcn', attn, v).reshape(c, h, w)
        out = jnp.einsum('chw,oc->ohw', out, proj_w) + proj_b[:, None, None]
        return grw * res + out

    def _shard(xs, ln_w, ln_b, qkv_w, qkv_b, dw_w, dw_b, proj_w, proj_b, temp, grw):
        # xs: [SLOTS, C, HH, WW]; params: [SLOTS, ...]
        return jax.vmap(_item)(xs, ln_w, ln_b, qkv_w, qkv_b, dw_w, dw_b,
                               proj_w, proj_b, temp, grw)

    run = jax.pmap(_shard, axis_name='cores')
    _jit_cache["run"] = run
    return run


def kernel(x, ln_w, ln_b, qkv_w, qkv_b, dw_w, dw_b, proj_w, proj_b,
           temperature, grw):
    run = _get_runner()

    # host-side sharding: [B,C,H,W] -> [T,B,C,HH,WW] (row-major tile order)
    tiles = x.reshape(B, C, 3, HH, 3, WW).transpose(2, 4, 0, 1, 3, 5) \
             .reshape(T, B, C, HH, WW)
    items_x = tiles.reshape(T * B, C, HH, WW)            # item j = (t=j//B, b=j%B)

    t_idx = np.arange(T * B) // B                         # tile index per item
    pad = N_CORES * SLOTS - T * B                         # 4 dummy slots
    t_idx = np.concatenate([t_idx, np.zeros(pad, np.int64)])
    items_x = np.concatenate([items_x, np.zeros((pad, C, HH, WW), items_x.dtype)])

    def sh(p):  # per-item param gather -> [N_CORES, SLOTS, ...]
        g = np.ascontiguousarray(p[t_idx])
        return g.reshape(N_CORES, SLOTS, *p.shape[1:])

    xs = items_x.reshape(N_CORES, SLOTS, C, HH, WW)
    out = run(xs, sh(ln_w), sh(ln_b), sh(qkv_w), sh(qkv_b), sh(dw_w),
              sh(dw_b), sh(proj_w), sh(proj_b), sh(temperature), sh(grw))
    out = np.asarray(out).reshape(N_CORES * SLOTS, C, HH, WW)[:T * B]

    # reassemble 3x3 grid
    out = out.reshape(3, 3, B, C, HH, WW).transpose(2, 3, 0, 4, 1, 5) \
             .reshape(B, C, H, W)
    return out.astype(x.dtype)



# revision 13
# speedup vs baseline: 1.0989x; 1.0989x over previous
"""Trainium2 Bass kernel for nn_Attention_33 (9-tile Restormer-style channel attention).

36 independent (tile, batch) items are sharded across 8 NeuronCores, 5 slots
each (4 zero dummies).  Per item the fused pipeline is:

  LayerNorm      mean removal folded into host-centered qkv weights; only the
                 per-token rstd is computed on device (mean/meansq via ones-
                 matmul broadcast, Rsqrt on ACT) and applied as xs = x * rstd.
  qkv 1x1 conv   PE matmuls, weights (ln_w-folded, centered, transposed) bf16.
  depthwise 3x3  the 6 dc=+-1 taps as PE diagonal-matmul PSUM accumulation,
                 the 3 dc=0 taps as DVE scalar_tensor_tensor FMAs in place.
  attention      q-hat/k-hat token chunks DMA-transposed, gram matrix
                 accumulated on PE; L2 norms + temperature folded in AFTER the
                 gram (rq per-partition, rk via a DRAM-roundtrip row broadcast);
                 masked softmax per 16-channel head block; P^T applied to v on
                 PE while v's depthwise conv streams chunk by chunk.
  projection     PE matmul + bias, then out = grw * x + proj (DVE fused FMA).

Compute dtype bf16 (PE 1 cyc/row), fp32 accumulation in PSUM; output returned
as bf16 and cast to fp32 on host.
"""
import numpy as np

B, C, H, W = 4, 128, 384, 384
HEADS = 8
T = 9
HH, WW = H // 3, W // 3            # 128, 128
N = HH * WW                        # 16384
N_CORES = 8
SLOTS = 5
NEG = -1e9
EPS_LN = 1e-5
EPS_NRM = 1e-12

# odd taps (dc = +-1) handled by PE diagonal matmuls; every PSUM element is
# covered by the two dr=0 taps, so start=True on the first clears the bank
PE_TAPS = [(0, -1), (0, 1), (-1, -1), (-1, 1), (1, -1), (1, 1)]

_cache = {}


# ---------------------------------------------------------------------------
# Bass program (one core: SLOTS items, per-slot weights from DRAM)
# ---------------------------------------------------------------------------

def _build_bass():
    import concourse.bass as bass
    import concourse.tile as tile
    from concourse import mybir
    from concourse.vector_clock import ScopedClock

    bf = mybir.dt.bfloat16
    f32 = mybir.dt.float32
    AF = mybir.ActivationFunctionType
    AL = mybir.AluOpType
    AX = mybir.AxisListType

    class TC(tile.TileContext):
        """Exit drain split into single-wait NOPs (neuronxcc rejects >2 waits)."""

        def _drain_and_barrier(self, tick_clock, wait_clock):
            nc = self.nc
            probe = mybir.InstNoOp(name="wait-probe", engine=mybir.EngineType.SP)
            wait_clock.add_sem_waits(probe, ScopedClock({None: tick_clock.global_clock}))
            by_name = {h.name: h for h in self.sems.allocated().values()}
            for w in probe.sync_info.on_wait:
                nc.sync.wait_ge(by_name[w.ant_name], w.wait_value)
            nc.sync.drain()
            nc.all_engine_barrier()
            popped = nc._tile_sem_poison_stack.pop()
            assert popped is self._sem_poison
            nc.clear_and_free_semaphores(list(self.sems.allocated().values()))
            nc.all_engine_barrier()

    nc = bass.Bass(enable_partition_id=False)

    x_d = nc.dram_tensor("x", [SLOTS, C, N], bf, kind="ExternalInput")
    wqkv_d = nc.dram_tensor("wqkv", [SLOTS, C, 3 * C], bf, kind="ExternalInput")
    bqkv_d = nc.dram_tensor("bqkv", [SLOTS, C, 3], f32, kind="ExternalInput")
    wdiag_d = nc.dram_tensor("wdiag", [SLOTS, C, 18 * C], bf, kind="ExternalInput")
    wdve_d = nc.dram_tensor("wdve", [SLOTS, C, 9], f32, kind="ExternalInput")
    bdw_d = nc.dram_tensor("bdw", [SLOTS, C, 3], f32, kind="ExternalInput")
    wproj_d = nc.dram_tensor("wproj", [SLOTS, C, C], bf, kind="ExternalInput")
    bproj_d = nc.dram_tensor("bproj", [SLOTS, C, 1], f32, kind="ExternalInput")
    rqs_d = nc.dram_tensor("rqs", [SLOTS, C, 1], f32, kind="ExternalInput")
    grw_d = nc.dram_tensor("grw", [SLOTS, C, 1], f32, kind="ExternalInput")
    ones_d = nc.dram_tensor("onesw", [C, C], bf, kind="ExternalInput")
    maskb_d = nc.dram_tensor("maskb", [C, C], f32, kind="ExternalInput")
    out_d = nc.dram_tensor("out", [SLOTS, C, N], bf, kind="ExternalOutput")

    def dw_pe_taps(ps, qv, wdiag_sb, g, k):
        """6 dc=+-1 taps for output rows 4k..4k+4 -> one accumulating psum."""
        ps_dw = ps.tile([C, 512], f32, tag="ps")
        psv = ps_dw.rearrange("p (h w) -> p h w", w=WW)
        r0, r1 = 4 * k, 4 * (k + 1)
        for j, (dr, dc) in enumerate(PE_TAPS):
            h_lo = max(r0, -dr)
            h_hi = min(r1, HH - dr) if dr > 0 else r1
            wlo, whi = max(0, -dc), WW - max(0, dc)
            rhs = qv[:, WW * (h_lo + dr): WW * (h_hi + dr)] \
                .rearrange("p (h w) -> p h w", w=WW)[:, :, wlo + dc: whi + dc]
            out_ap = psv[:, h_lo - r0: h_hi - r0, wlo:whi]
            lhsT = wdiag_sb[:, C * (6 * g + j): C * (6 * g + j + 1)]
            nc.tensor.matmul(out_ap, lhsT, rhs, start=(j == 0),
                             stop=(j == len(PE_TAPS) - 1))
        return ps_dw

    def dw_dve_taps(qv, ch, wdve_sb, g, k):
        """3 dc=0 taps as in-place fused FMAs on the evicted chunk."""
        r0, r1 = 4 * k, 4 * (k + 1)
        for dr in (-1, 0, 1):
            h_lo = max(r0, -dr)
            h_hi = min(r1, HH - dr) if dr > 0 else r1
            in0 = qv[:, WW * (h_lo + dr): WW * (h_hi + dr)]
            dst = ch[:, WW * (h_lo - r0): WW * (h_hi - r0)]
            nc.vector.scalar_tensor_tensor(
                out=dst, in0=in0,
                scalar=wdve_sb[:, 3 * g + dr + 1: 3 * g + dr + 2],
                in1=dst, op0=AL.mult, op1=AL.add)

    def _split_waits(maxw=1):
        """neuronxcc rejects instructions with more than ~2 sync waits; hoist
        the excess onto same-engine NOPs inserted just before the offender."""
        import bass_rust
        cnt = 0
        for blk in nc.m.functions[0].blocks:
            insts = blk.instructions
            i = 0
            while i < len(insts):
                inst = insts[i]
                si = inst.sync_info
                if si is not None and len(si.on_wait) > maxw:
                    waits = list(si.on_wait)
                    extra, keep = waits[:-maxw], waits[-maxw:]
                    nops = []
                    for j in range(0, len(extra), maxw):
                        cnt += 1
                        nop = mybir.InstNoOp(name=f"wsplit-{cnt}",
                                             engine=inst.engine)
                        nop.sync_info = bass_rust.SyncInfo(
                            on_wait=extra[j:j + maxw], on_update=[])
                        nops.append(nop)
                    inst.sync_info = bass_rust.SyncInfo(
                        on_wait=keep, on_update=list(si.on_update))
                    insts[i:i] = nops
                    i += len(nops)
                i += 1

    from contextlib import ExitStack
    with ExitStack() as ctx:
        tc = ctx.enter_context(TC(nc))
        pool = lambda name, bufs, **kw: ctx.enter_context(
            tc.tile_pool(name=name, bufs=bufs, **kw))
        wconst = pool("wconst", 1)
        wslot = pool("wslot", 1)
        xin = pool("xin", 3)
        x2p = pool("x2p", 2)
        varp = pool("varp", 1)
        acp = pool("acp", 2)
        xsp = pool("xsp", 1)
        qkvp = pool("qkvp", 1)
        qtp = pool("qtp", 1)
        dwch = pool("dwch", 3)
        ktch = pool("ktch", 2)
        attp = pool("attp", 2)
        prjp = pool("prjp", 2)
        outp = pool("outp", 2)
        vecp = pool("vecp", 1)
        dramp = pool("dramp", 2, space="DRAM")
        ps = pool("ps", 4, space="PSUM")
        psg = pool("psg", 1, space="PSUM")

        ones_sb = wconst.tile([C, C], bf)          # all entries 1/128
        nc.gpsimd.dma_start(out=ones_sb, in_=ones_d[:, :])
        maskb_sb = wconst.tile([C, C], f32)        # 0 on head blocks, -1e9 off
        nc.gpsimd.dma_start(out=maskb_sb, in_=maskb_d[:, :])
        eps_sb = wconst.tile([C, 1], f32)
        nc.vector.memset(eps_sb, EPS_LN)

        for s in range(SLOTS):
            # ---- per-slot weights -------------------------------------
            wb = wslot.tile([C, 3 * C + 18 * C + C], bf, tag="wb")
            wqkv_sb = wb[:, 0:3 * C]
            wdiag_sb = wb[:, 3 * C:21 * C]
            wproj_sb = wb[:, 21 * C:22 * C]
            nc.gpsimd.dma_start(out=wqkv_sb, in_=wqkv_d[s])
            nc.gpsimd.dma_start(out=wdiag_sb, in_=wdiag_d[s])
            nc.gpsimd.dma_start(out=wproj_sb, in_=wproj_d[s])
            wf = wslot.tile([C, 18], f32, tag="wf")
            bqkv_sb, wdve_sb = wf[:, 0:3], wf[:, 3:12]
            bdw_sb, bproj_sb = wf[:, 12:15], wf[:, 15:16]
            rqs_sb, grw_sb = wf[:, 16:17], wf[:, 17:18]
            nc.gpsimd.dma_start(out=bqkv_sb, in_=bqkv_d[s])
            nc.gpsimd.dma_start(out=wdve_sb, in_=wdve_d[s])
            nc.gpsimd.dma_start(out=bdw_sb, in_=bdw_d[s])
            nc.gpsimd.dma_start(out=bproj_sb, in_=bproj_d[s])
            nc.gpsimd.dma_start(out=rqs_sb, in_=rqs_d[s])
            nc.gpsimd.dma_start(out=grw_sb, in_=grw_d[s])

            vec = vecp.tile([C, 96], f32, tag="vec")
            acc_q, acc_k = vec[:, 0:32], vec[:, 32:64]

            # ---- phase 1: LN rstd + xs = x * rstd ---------------------
            xs_sb = xsp.tile([C, N], bf, tag="xs")
            for j in range(8):
                xc = xin.tile([C, 2048], bf, tag="xc")
                nc.gpsimd.dma_start(out=xc, in_=x_d[s, :, 2048 * j:2048 * (j + 1)])
                x2c = x2p.tile([C, 2048], bf, tag="x2c")
                nc.scalar.activation(out=x2c, in_=xc, func=AF.Square)
                vc = varp.tile([C, 2048], f32, tag="vc")
                for k in range(4):
                    sl = slice(512 * k, 512 * (k + 1))
                    mu_ps = ps.tile([C, 512], f32, tag="ps")
                    nc.tensor.matmul(mu_ps, ones_sb, xc[:, sl], start=True, stop=True)
                    s2_ps = ps.tile([C, 512], f32, tag="ps")
                    nc.tensor.matmul(s2_ps, ones_sb, x2c[:, sl], start=True, stop=True)
                    musq = x2p.tile([C, 512], f32, tag="musq")
                    nc.scalar.activation(out=musq, in_=mu_ps, func=AF.Square)
                    nc.vector.scalar_tensor_tensor(
                        out=vc[:, sl], in0=s2_ps, scalar=1.0, in1=musq,
                        op0=AL.mult, op1=AL.subtract)
                ac = acp.tile([C, 2048], bf, tag="ac")
                nc.scalar.activation(out=vc, in_=vc, func=AF.Ln, bias=eps_sb)
                nc.scalar.activation(out=ac, in_=vc, func=AF.Exp, scale=-0.5)
                nc.vector.tensor_mul(out=xs_sb[:, 2048 * j:2048 * (j + 1)],
                                     in0=xc, in1=ac)

            # ---- phase 2: q then k — qkv matmul, dwconv, transpose ----
            qT = qtp.tile([C, N], bf, tag="qT")
            G = psg.tile([C, C], f32, tag="G")
            for g in range(2):
                qv = qkvp.tile([C, N], bf, tag="qv")
                lhsT = wqkv_sb[:, C * g:C * (g + 1)]
                for k in range(32):
                    q_ps = ps.tile([C, 512], f32, tag="ps")
                    nc.tensor.matmul(q_ps, lhsT, xs_sb[:, 512 * k:512 * (k + 1)],
                                     start=True, stop=True)
                    dst = qv[:, 512 * k:512 * (k + 1)]
                    if k % 2 == 0:
                        nc.scalar.add(out=dst, in_=q_ps, add=bqkv_sb[:, g:g + 1])
                    else:
                        nc.vector.tensor_scalar_add(out=dst, in0=q_ps,
                                                    scalar1=bqkv_sb[:, g:g + 1])
                acc = acc_q if g == 0 else acc_k
                for k in range(32):
                    ch = dwch.tile([C, 512], bf, tag="ch")
                    ps_dw = dw_pe_taps(ps, qv, wdiag_sb, g, k)
                    nc.scalar.add(out=ch, in_=ps_dw, add=bdw_sb[:, g:g + 1])
                    dw_dve_taps(qv, ch, wdve_sb, g, k)
                    if g == 0:
                        for t_ in range(4):
                            nc.sync.dma_start_transpose(
                                out=qT[:, 512 * k + 128 * t_: 512 * k + 128 * (t_ + 1)],
                                in_=ch[:, 128 * t_:128 * (t_ + 1)])
                    else:
                        kT = ktch.tile([C, 512], bf, tag="kT")
                        for t_ in range(4):
                            nc.sync.dma_start_transpose(
                                out=kT[:, 128 * t_:128 * (t_ + 1)],
                                in_=ch[:, 128 * t_:128 * (t_ + 1)])
                        for t_ in range(4):
                            sl = slice(512 * k + 128 * t_, 512 * k + 128 * (t_ + 1))
                            nc.tensor.matmul(G, qT[:, sl], kT[:, 128 * t_:128 * (t_ + 1)],
                                             start=(k == 0 and t_ == 0),
                                             stop=(k == 31 and t_ == 3))
                    # ||.||^2 accumulation; chunk is dead after this
                    nc.scalar.activation(out=ch, in_=ch, func=AF.Square,
                                         accum_out=acc[:, k:k + 1])

            # ---- phase 3: norms, masked softmax, P^T ------------------
            sq = vec[:, 64:65]
            nc.vector.tensor_reduce(out=sq, in_=acc_q, axis=AX.X, op=AL.add)
            nc.vector.tensor_scalar_max(out=sq, in0=sq, scalar1=EPS_NRM * EPS_NRM)
            nc.scalar.activation(out=sq, in_=sq, func=AF.Ln)
            rq = vec[:, 65:66]
            nc.scalar.activation(out=rq, in_=sq, func=AF.Exp, scale=-0.5)
            nc.vector.tensor_mul(out=rq, in0=rq, in1=rqs_sb)   # fold temperature
            sk = vec[:, 66:67]
            nc.vector.tensor_reduce(out=sk, in_=acc_k, axis=AX.X, op=AL.add)
            nc.vector.tensor_scalar_max(out=sk, in0=sk, scalar1=EPS_NRM * EPS_NRM)
            nc.scalar.activation(out=sk, in_=sk, func=AF.Ln)
            rk = vec[:, 67:68]
            nc.scalar.activation(out=rk, in_=sk, func=AF.Exp, scale=-0.5)
            rk128 = vecp.tile([C, 1], bf, tag="rk128")
            nc.vector.tensor_scalar_mul(out=rk128, in0=rk, scalar1=128.0)
            rkt = dramp.tile([C, 1], bf, tag="rkt")
            nc.gpsimd.dma_start(out=rkt, in_=rk128)
            rkrow = vecp.tile([1, C], bf, tag="rkrow")
            nc.gpsimd.dma_start(out=rkrow, in_=rkt.rearrange("c one -> one c"))
            rkb_ps = psg.tile([C, C], f32, tag="rkb")
            nc.tensor.matmul(rkb_ps, ones_sb[0:1, :], rkrow, start=True, stop=True)
            rkb = vecp.tile([C, C], bf, tag="rkb_sb")
            nc.scalar.copy(out=rkb, in_=rkb_ps)

            Lg = vecp.tile([C, 2 * C], f32, tag="Lg")
            L_sb, Pexp = Lg[:, 0:C], Lg[:, C:2 * C]
            # L = (G * rq[c]) * rk[d] * temp, then + head-block mask
            nc.vector.scalar_tensor_tensor(out=L_sb, in0=G, scalar=rq, in1=rkb,
                                           op0=AL.mult, op1=AL.mult)
            nc.vector.tensor_add(out=L_sb, in0=L_sb, in1=maskb_sb)
            mx = vec[:, 68:69]
            nc.vector.tensor_reduce(out=mx, in_=L_sb, axis=AX.X, op=AL.max)
            nmx = vec[:, 69:70]
            nc.vector.tensor_scalar_mul(out=nmx, in0=mx, scalar1=-1.0)
            nc.scalar.activation(out=Pexp, in_=L_sb, func=AF.Exp, bias=nmx)
            den = vec[:, 70:71]
            nc.vector.tensor_reduce(out=den, in_=Pexp, axis=AX.X, op=AL.add)
            rden = vec[:, 71:72]
            nc.vector.reciprocal(out=rden, in_=den)
            Pg = vecp.tile([C, 2 * C], bf, tag="Pg")
            P_sb, Pt_sb = Pg[:, 0:C], Pg[:, C:2 * C]
            nc.vector.tensor_scalar_mul(out=P_sb, in0=Pexp, scalar1=rden)
            nc.sync.dma_start_transpose(out=Pt_sb, in_=P_sb)

            # ---- phase 4: v stream -> attn apply -> proj -> residual --
            qv = qkvp.tile([C, N], bf, tag="qv")
            lhsT = wqkv_sb[:, 2 * C:3 * C]
            for k in range(32):
                q_ps = ps.tile([C, 512], f32, tag="ps")
                nc.tensor.matmul(q_ps, lhsT, xs_sb[:, 512 * k:512 * (k + 1)],
                                 start=True, stop=True)
                dst = qv[:, 512 * k:512 * (k + 1)]
                if k % 2 == 0:
                    nc.scalar.add(out=dst, in_=q_ps, add=bqkv_sb[:, 2:3])
                else:
                    nc.vector.tensor_scalar_add(out=dst, in0=q_ps,
                                                scalar1=bqkv_sb[:, 2:3])
            for j in range(8):
                res = xin.tile([C, 2048], bf, tag="xc")
                nc.gpsimd.dma_start(out=res, in_=x_d[s, :, 2048 * j:2048 * (j + 1)])
                oc = outp.tile([C, 2048], bf, tag="oc")
                for kk in range(4):
                    k = 4 * j + kk
                    ch = dwch.tile([C, 512], bf, tag="ch")
                    ps_dw = dw_pe_taps(ps, qv, wdiag_sb, 2, k)
                    nc.scalar.add(out=ch, in_=ps_dw, add=bdw_sb[:, 2:3])
                    dw_dve_taps(qv, ch, wdve_sb, 2, k)
                    av_ps = ps.tile([C, 512], f32, tag="ps")
                    nc.tensor.matmul(av_ps, Pt_sb, ch, start=True, stop=True)
                    att = attp.tile([C, 512], bf, tag="att")
                    nc.scalar.copy(out=att, in_=av_ps)
                    pj_ps = ps.tile([C, 512], f32, tag="ps")
                    nc.tensor.matmul(pj_ps, wproj_sb, att, start=True, stop=True)
                    pj = prjp.tile([C, 512], bf, tag="pj")
                    nc.scalar.add(out=pj, in_=pj_ps, add=bproj_sb)
                    ssl = slice(512 * kk, 512 * (kk + 1))
                    nc.vector.scalar_tensor_tensor(
                        out=oc[:, ssl], in0=res[:, ssl], scalar=grw_sb, in1=pj,
                        op0=AL.mult, op1=AL.add)
                nc.gpsimd.dma_start(out=out_d[s, :, 2048 * j:2048 * (j + 1)], in_=oc)

    _split_waits()
    return nc


# ---------------------------------------------------------------------------
# Host side: weight prep, sharding, cached PJRT runner
# ---------------------------------------------------------------------------

def _prep_core_inputs(x, ln_w, ln_b, qkv_w, qkv_b, dw_w, dw_b, proj_w, proj_b,
                      temperature, grw):
    import ml_dtypes
    bf = ml_dtypes.bfloat16

    tiles = x.reshape(B, C, 3, HH, 3, WW).transpose(2, 4, 0, 1, 3, 5) \
             .reshape(T, B, C, HH, WW)
    items_x = tiles.reshape(T * B, C, N)
    pad = N_CORES * SLOTS - T * B
    items_x = np.concatenate(
        [items_x, np.zeros((pad, C, N), items_x.dtype)]).astype(bf)
    t_idx = np.concatenate([np.arange(T * B) // B, np.zeros(pad, np.int64)])

    WQKV = np.zeros((T, C, 3 * C), np.float32)
    BQKV = np.zeros((T, C, 3), np.float32)
    WDIAG = np.zeros((T, C, 18 * C), np.float32)
    WDVE = np.zeros((T, C, 9), np.float32)
    BDW = np.zeros((T, C, 3), np.float32)
    WPROJ = np.zeros((T, C, C), np.float32)
    for t in range(T):
        Wp = qkv_w[t] * ln_w[t][None, :]           # [384, 128]
        Wc = Wp - Wp.mean(axis=1, keepdims=True)   # fold LN mean removal
        bp = qkv_b[t] + qkv_w[t] @ ln_b[t]         # [384]
        for g in range(3):
            WQKV[t, :, C * g:C * (g + 1)] = Wc[C * g:C * (g + 1), :].T
            BQKV[t, :, g] = bp[C * g:C * (g + 1)]
            BDW[t, :, g] = dw_b[t, C * g:C * (g + 1)]
            for j, (dr, dc) in enumerate(PE_TAPS):
                d = np.diag(dw_w[t, C * g:C * (g + 1), 0, dr + 1, dc + 1])
                WDIAG[t, :, C * (6 * g + j):C * (6 * g + j + 1)] = d
            for dr in (-1, 0, 1):
                WDVE[t, :, 3 * g + dr + 1] = dw_w[t, C * g:C * (g + 1), 0, dr + 1, 1]
        WPROJ[t] = proj_w[t].T

    RQS = np.repeat(temperature, C // HEADS, axis=1)[:, :, None]  # [T, C, 1]
    GRW = np.broadcast_to(np.asarray(grw)[:, None, None], (T, C, 1))

    mask = np.full((C, C), NEG, np.float32)
    for h in range(HEADS):
        mask[16 * h:16 * (h + 1), 16 * h:16 * (h + 1)] = 0.0
    ones = np.full((C, C), 1.0 / C, np.float32)

    in_maps = []
    for c in range(N_CORES):
        ts = t_idx[c * SLOTS:(c + 1) * SLOTS]
        in_maps.append(dict(
            x=np.ascontiguousarray(items_x[c * SLOTS:(c + 1) * SLOTS]),
            wqkv=WQKV[ts].astype(bf),
            bqkv=np.ascontiguousarray(BQKV[ts]),
            wdiag=WDIAG[ts].astype(bf),
            wdve=np.ascontiguousarray(WDVE[ts]),
            bdw=np.ascontiguousarray(BDW[ts]),
            wproj=WPROJ[ts].astype(bf),
            bproj=np.ascontiguousarray(proj_b[ts][:, :, None]).astype(np.float32),
            rqs=np.ascontiguousarray(RQS[ts]).astype(np.float32),
            grw=np.ascontiguousarray(GRW[ts]).astype(np.float32),
            onesw=ones.astype(bf),
            maskb=mask,
        ))
    return in_maps


def _get_runner():
    if "run" in _cache:
        return _cache["run"]

    import jax
    from jax.sharding import Mesh, PartitionSpec
    try:
        from jax.experimental.shard_map import shard_map
    except ImportError:
        from jax import shard_map
    from concourse import bass2jax, mybir

    nc = _build_bass()
    bass2jax.install_neuronx_cc_hook()

    in_names, out_names, out_avals, zero_outs = [], [], [], []
    for alloc in nc.m.functions[0].allocations:
        if not isinstance(alloc, mybir.MemoryLocationSet):
            continue
        name = alloc.memorylocations[0].name
        if alloc.kind == "ExternalInput":
            in_names.append(name)
        elif alloc.kind == "ExternalOutput":
            out_names.append(name)
            shape = tuple(alloc.tensor_shape)
            dtype = mybir.dt.np(alloc.dtype)
            out_avals.append(jax.core.ShapedArray(shape, dtype))
            zero_outs.append(np.zeros(shape, dtype))
    n_params = len(in_names)
    all_names = in_names + out_names
    donate = tuple(range(n_params, n_params + len(out_names)))

    def _body(*args):
        outs = bass2jax._bass_exec_p.bind(
            *args,
            out_avals=tuple(out_avals),
            in_names=tuple(all_names),
            out_names=tuple(out_names),
            lowering_input_output_aliases=(),
            sim_require_finite=True,
            sim_require_nnan=True,
            nc=nc,
        )
        return tuple(outs)

    devices = jax.devices()[:N_CORES]
    mesh = Mesh(np.asarray(devices), ("core",))
    in_specs = (PartitionSpec("core"),) * (n_params + len(out_names))
    out_specs = (PartitionSpec("core"),) * len(out_names)
    sharded = jax.jit(
        shard_map(_body, mesh=mesh, in_specs=in_specs, out_specs=out_specs,
                  check_rep=False),
        donate_argnums=donate, keep_unused=True)

    run = dict(fn=sharded, in_names=in_names, out_names=out_names,
               zero_outs=zero_outs, n_params=n_params)
    _cache["run"] = run
    return run


def _concat_inputs(run, in_maps):
    return [np.concatenate([m[name] for m in in_maps], axis=0)
            for name in run["in_names"]]


def _fresh_zeros(run):
    return [np.zeros((N_CORES * z.shape[0],) + z.shape[1:], z.dtype)
            for z in run["zero_outs"]]


def kernel(x, ln_w, ln_b, qkv_w, qkv_b, dw_w, dw_b, proj_w, proj_b,
           temperature, grw):
    run = _get_runner()
    in_maps = _prep_core_inputs(
        np.asarray(x, np.float32), np.asarray(ln_w), np.asarray(ln_b),
        np.asarray(qkv_w), np.asarray(qkv_b), np.asarray(dw_w),
        np.asarray(dw_b), np.asarray(proj_w), np.asarray(proj_b),
        np.asarray(temperature), np.asarray(grw))
    out_arrs = run["fn"](*_concat_inputs(run, in_maps), *_fresh_zeros(run))
    out = np.asarray(out_arrs[0]).astype(np.float32)    # [40, C, N]
    out = out.reshape(N_CORES * SLOTS, C, HH, WW)[:T * B]
    out = out.reshape(3, 3, B, C, HH, WW).transpose(2, 3, 0, 4, 1, 5) \
             .reshape(B, C, H, W)
    return np.ascontiguousarray(out, dtype=np.float32)


# revision 14
# speedup vs baseline: 3.2706x; 2.9763x over previous
"""Trainium2 Bass kernel for nn_Attention_33 (9-tile Restormer-style channel attention).

36 independent (tile, batch) items are sharded across 8 NeuronCores, 5 slots
each (4 zero dummies).  Per item the fused pipeline is:

  LayerNorm      mean removal folded into host-centered qkv weights; only the
                 per-token rstd is computed on device (mean/meansq via ones-
                 matmul broadcast, Rsqrt on ACT) and applied as xs = x * rstd.
  qkv 1x1 conv   PE matmuls, weights (ln_w-folded, centered, transposed) bf16.
  depthwise 3x3  the 6 dc=+-1 taps as PE diagonal-matmul PSUM accumulation,
                 the 3 dc=0 taps as DVE scalar_tensor_tensor FMAs in place.
  attention      q-hat/k-hat token chunks DMA-transposed, gram matrix
                 accumulated on PE; L2 norms + temperature folded in AFTER the
                 gram (rq per-partition, rk via a DRAM-roundtrip row broadcast);
                 masked softmax per 16-channel head block; P^T applied to v on
                 PE while v's depthwise conv streams chunk by chunk.
  projection     PE matmul + bias, then out = grw * x + proj (DVE fused FMA).

Compute dtype bf16 (PE 1 cyc/row), fp32 accumulation in PSUM; output returned
as bf16 and cast to fp32 on host.
"""
import numpy as np

B, C, H, W = 4, 128, 384, 384
HEADS = 8
T = 9
HH, WW = H // 3, W // 3            # 128, 128
N = HH * WW                        # 16384
N_CORES = 8
SLOTS = 5
NEG = -1e9
EPS_LN = 1e-5
EPS_NRM = 1e-12

# odd taps (dc = +-1) handled by PE diagonal matmuls; every PSUM element is
# covered by the two dr=0 taps, so start=True on the first clears the bank
PE_TAPS = [(0, -1), (0, 1), (-1, -1), (-1, 1), (1, -1), (1, 1)]

_cache = {}


# ---------------------------------------------------------------------------
# Bass program (one core: SLOTS items, per-slot weights from DRAM)
# ---------------------------------------------------------------------------

def _build_bass():
    import concourse.bass as bass
    import concourse.tile as tile
    from concourse import mybir
    from concourse.vector_clock import ScopedClock

    bf = mybir.dt.bfloat16
    f32 = mybir.dt.float32
    AF = mybir.ActivationFunctionType
    AL = mybir.AluOpType
    AX = mybir.AxisListType

    class TC(tile.TileContext):
        """Exit drain split into single-wait NOPs (neuronxcc rejects >2 waits)."""

        def _drain_and_barrier(self, tick_clock, wait_clock):
            nc = self.nc
            probe = mybir.InstNoOp(name="wait-probe", engine=mybir.EngineType.SP)
            wait_clock.add_sem_waits(probe, ScopedClock({None: tick_clock.global_clock}))
            by_name = {h.name: h for h in self.sems.allocated().values()}
            for w in probe.sync_info.on_wait:
                nc.sync.wait_ge(by_name[w.ant_name], w.wait_value)
            nc.sync.drain()
            nc.all_engine_barrier()
            popped = nc._tile_sem_poison_stack.pop()
            assert popped is self._sem_poison
            nc.clear_and_free_semaphores(list(self.sems.allocated().values()))
            nc.all_engine_barrier()

    nc = bass.Bass(enable_partition_id=False)

    x_d = nc.dram_tensor("x", [SLOTS, C, N], bf, kind="ExternalInput")
    wqkv_d = nc.dram_tensor("wqkv", [SLOTS, C, 3 * C], bf, kind="ExternalInput")
    bqkv_d = nc.dram_tensor("bqkv", [SLOTS, C, 3], f32, kind="ExternalInput")
    wdiag_d = nc.dram_tensor("wdiag", [SLOTS, C, 18 * C], bf, kind="ExternalInput")
    wdve_d = nc.dram_tensor("wdve", [SLOTS, C, 9], f32, kind="ExternalInput")
    bdw_d = nc.dram_tensor("bdw", [SLOTS, C, 3], f32, kind="ExternalInput")
    wproj_d = nc.dram_tensor("wproj", [SLOTS, C, C], bf, kind="ExternalInput")
    bproj_d = nc.dram_tensor("bproj", [SLOTS, C, 1], f32, kind="ExternalInput")
    rqs_d = nc.dram_tensor("rqs", [SLOTS, C, 1], f32, kind="ExternalInput")
    grw_d = nc.dram_tensor("grw", [SLOTS, C, 1], f32, kind="ExternalInput")
    ones_d = nc.dram_tensor("onesw", [C, C], bf, kind="ExternalInput")
    maskb_d = nc.dram_tensor("maskb", [C, C], f32, kind="ExternalInput")
    out_d = nc.dram_tensor("out", [SLOTS, C, N], bf, kind="ExternalOutput")

    def dw_pe_taps(ps, qv, wdiag_sb, g, k):
        """6 dc=+-1 taps for output rows 4k..4k+4 -> one accumulating psum."""
        ps_dw = ps.tile([C, 512], f32, tag="ps")
        psv = ps_dw.rearrange("p (h w) -> p h w", w=WW)
        r0, r1 = 4 * k, 4 * (k + 1)
        for j, (dr, dc) in enumerate(PE_TAPS):
            h_lo = max(r0, -dr)
            h_hi = min(r1, HH - dr) if dr > 0 else r1
            wlo, whi = max(0, -dc), WW - max(0, dc)
            rhs = qv[:, WW * (h_lo + dr): WW * (h_hi + dr)] \
                .rearrange("p (h w) -> p h w", w=WW)[:, :, wlo + dc: whi + dc]
            out_ap = psv[:, h_lo - r0: h_hi - r0, wlo:whi]
            lhsT = wdiag_sb[:, C * (6 * g + j): C * (6 * g + j + 1)]
            nc.tensor.matmul(out_ap, lhsT, rhs, start=(j == 0),
                             stop=(j == len(PE_TAPS) - 1))
        return ps_dw

    def dw_dve_taps(qv, ch, wdve_sb, g, k):
        """3 dc=0 taps as in-place fused FMAs on the evicted chunk."""
        r0, r1 = 4 * k, 4 * (k + 1)
        for dr in (-1, 0, 1):
            h_lo = max(r0, -dr)
            h_hi = min(r1, HH - dr) if dr > 0 else r1
            in0 = qv[:, WW * (h_lo + dr): WW * (h_hi + dr)]
            dst = ch[:, WW * (h_lo - r0): WW * (h_hi - r0)]
            nc.vector.scalar_tensor_tensor(
                out=dst, in0=in0,
                scalar=wdve_sb[:, 3 * g + dr + 1: 3 * g + dr + 2],
                in1=dst, op0=AL.mult, op1=AL.add)

    def _split_waits(maxw=1):
        """neuronxcc rejects instructions with more than ~2 sync waits; hoist
        the excess onto same-engine NOPs inserted just before the offender."""
        import bass_rust
        cnt = 0
        for blk in nc.m.functions[0].blocks:
            insts = blk.instructions
            i = 0
            while i < len(insts):
                inst = insts[i]
                si = inst.sync_info
                if si is not None and len(si.on_wait) > maxw:
                    waits = list(si.on_wait)
                    extra, keep = waits[:-maxw], waits[-maxw:]
                    nops = []
                    for j in range(0, len(extra), maxw):
                        cnt += 1
                        nop = mybir.InstNoOp(name=f"wsplit-{cnt}",
                                             engine=inst.engine)
                        nop.sync_info = bass_rust.SyncInfo(
                            on_wait=extra[j:j + maxw], on_update=[])
                        nops.append(nop)
                    inst.sync_info = bass_rust.SyncInfo(
                        on_wait=keep, on_update=list(si.on_update))
                    insts[i:i] = nops
                    i += len(nops)
                i += 1

    from contextlib import ExitStack
    with ExitStack() as ctx:
        tc = ctx.enter_context(TC(nc))
        pool = lambda name, bufs, **kw: ctx.enter_context(
            tc.tile_pool(name=name, bufs=bufs, **kw))
        wconst = pool("wconst", 1)
        wslot = pool("wslot", 1)
        xin = pool("xin", 3)
        x2p = pool("x2p", 2)
        varp = pool("varp", 1)
        acp = pool("acp", 2)
        xsp = pool("xsp", 1)
        qkvp = pool("qkvp", 1)
        qtp = pool("qtp", 1)
        dwch = pool("dwch", 3)
        ktch = pool("ktch", 2)
        attp = pool("attp", 2)
        prjp = pool("prjp", 2)
        outp = pool("outp", 2)
        vecp = pool("vecp", 1)
        dramp = pool("dramp", 2, space="DRAM")
        ps = pool("ps", 4, space="PSUM")
        psg = pool("psg", 1, space="PSUM")

        ones_sb = wconst.tile([C, C], bf)          # all entries 1/128
        nc.gpsimd.dma_start(out=ones_sb, in_=ones_d[:, :])
        maskb_sb = wconst.tile([C, C], f32)        # 0 on head blocks, -1e9 off
        nc.gpsimd.dma_start(out=maskb_sb, in_=maskb_d[:, :])
        eps_sb = wconst.tile([C, 1], f32)
        nc.vector.memset(eps_sb, EPS_LN)

        for s in range(SLOTS):
            # ---- per-slot weights -------------------------------------
            wb = wslot.tile([C, 3 * C + 18 * C + C], bf, tag="wb")
            wqkv_sb = wb[:, 0:3 * C]
            wdiag_sb = wb[:, 3 * C:21 * C]
            wproj_sb = wb[:, 21 * C:22 * C]
            nc.gpsimd.dma_start(out=wqkv_sb, in_=wqkv_d[s])
            nc.gpsimd.dma_start(out=wdiag_sb, in_=wdiag_d[s])
            nc.gpsimd.dma_start(out=wproj_sb, in_=wproj_d[s])
            wf = wslot.tile([C, 18], f32, tag="wf")
            bqkv_sb, wdve_sb = wf[:, 0:3], wf[:, 3:12]
            bdw_sb, bproj_sb = wf[:, 12:15], wf[:, 15:16]
            rqs_sb, grw_sb = wf[:, 16:17], wf[:, 17:18]
            nc.gpsimd.dma_start(out=bqkv_sb, in_=bqkv_d[s])
            nc.gpsimd.dma_start(out=wdve_sb, in_=wdve_d[s])
            nc.gpsimd.dma_start(out=bdw_sb, in_=bdw_d[s])
            nc.gpsimd.dma_start(out=bproj_sb, in_=bproj_d[s])
            nc.gpsimd.dma_start(out=rqs_sb, in_=rqs_d[s])
            nc.gpsimd.dma_start(out=grw_sb, in_=grw_d[s])

            vec = vecp.tile([C, 96], f32, tag="vec")
            acc_q, acc_k = vec[:, 0:32], vec[:, 32:64]

            # ---- phase 1: LN rstd + xs = x * rstd ---------------------
            xs_sb = xsp.tile([C, N], bf, tag="xs")
            for j in range(8):
                xc = xin.tile([C, 2048], bf, tag="xc")
                nc.gpsimd.dma_start(out=xc, in_=x_d[s, :, 2048 * j:2048 * (j + 1)])
                x2c = x2p.tile([C, 2048], bf, tag="x2c")
                nc.scalar.activation(out=x2c, in_=xc, func=AF.Square)
                vc = varp.tile([C, 2048], f32, tag="vc")
                for k in range(4):
                    sl = slice(512 * k, 512 * (k + 1))
                    mu_ps = ps.tile([C, 512], f32, tag="ps")
                    nc.tensor.matmul(mu_ps, ones_sb, xc[:, sl], start=True, stop=True)
                    s2_ps = ps.tile([C, 512], f32, tag="ps")
                    nc.tensor.matmul(s2_ps, ones_sb, x2c[:, sl], start=True, stop=True)
                    musq = x2p.tile([C, 512], f32, tag="musq")
                    nc.scalar.activation(out=musq, in_=mu_ps, func=AF.Square)
                    nc.vector.scalar_tensor_tensor(
                        out=vc[:, sl], in0=s2_ps, scalar=1.0, in1=musq,
                        op0=AL.mult, op1=AL.subtract)
                ac = acp.tile([C, 2048], bf, tag="ac")
                nc.scalar.activation(out=vc, in_=vc, func=AF.Ln, bias=eps_sb)
                nc.scalar.activation(out=ac, in_=vc, func=AF.Exp, scale=-0.5)
                nc.vector.tensor_mul(out=xs_sb[:, 2048 * j:2048 * (j + 1)],
                                     in0=xc, in1=ac)

            # ---- phase 2: q then k — qkv matmul, dwconv, transpose ----
            qT = qtp.tile([C, N], bf, tag="qT")
            G = psg.tile([C, C], f32, tag="G")
            for g in range(2):
                qv = qkvp.tile([C, N], bf, tag="qv")
                lhsT = wqkv_sb[:, C * g:C * (g + 1)]
                for k in range(32):
                    q_ps = ps.tile([C, 512], f32, tag="ps")
                    nc.tensor.matmul(q_ps, lhsT, xs_sb[:, 512 * k:512 * (k + 1)],
                                     start=True, stop=True)
                    dst = qv[:, 512 * k:512 * (k + 1)]
                    if k % 2 == 0:
                        nc.scalar.add(out=dst, in_=q_ps, add=bqkv_sb[:, g:g + 1])
                    else:
                        nc.vector.tensor_scalar_add(out=dst, in0=q_ps,
                                                    scalar1=bqkv_sb[:, g:g + 1])
                acc = acc_q if g == 0 else acc_k
                for k in range(32):
                    ch = dwch.tile([C, 512], bf, tag="ch")
                    ps_dw = dw_pe_taps(ps, qv, wdiag_sb, g, k)
                    nc.scalar.add(out=ch, in_=ps_dw, add=bdw_sb[:, g:g + 1])
                    dw_dve_taps(qv, ch, wdve_sb, g, k)
                    if g == 0:
                        for t_ in range(4):
                            nc.sync.dma_start_transpose(
                                out=qT[:, 512 * k + 128 * t_: 512 * k + 128 * (t_ + 1)],
                                in_=ch[:, 128 * t_:128 * (t_ + 1)])
                    else:
                        kT = ktch.tile([C, 512], bf, tag="kT")
                        for t_ in range(4):
                            nc.sync.dma_start_transpose(
                                out=kT[:, 128 * t_:128 * (t_ + 1)],
                                in_=ch[:, 128 * t_:128 * (t_ + 1)])
                        for t_ in range(4):
                            sl = slice(512 * k + 128 * t_, 512 * k + 128 * (t_ + 1))
                            nc.tensor.matmul(G, qT[:, sl], kT[:, 128 * t_:128 * (t_ + 1)],
                                             start=(k == 0 and t_ == 0),
                                             stop=(k == 31 and t_ == 3))
                    # ||.||^2 accumulation; chunk is dead after this
                    nc.scalar.activation(out=ch, in_=ch, func=AF.Square,
                                         accum_out=acc[:, k:k + 1])

            # ---- phase 3: norms, masked softmax, P^T ------------------
            sq = vec[:, 64:65]
            nc.vector.tensor_reduce(out=sq, in_=acc_q, axis=AX.X, op=AL.add)
            nc.vector.tensor_scalar_max(out=sq, in0=sq, scalar1=EPS_NRM * EPS_NRM)
            nc.scalar.activation(out=sq, in_=sq, func=AF.Ln)
            rq = vec[:, 65:66]
            nc.scalar.activation(out=rq, in_=sq, func=AF.Exp, scale=-0.5)
            nc.vector.tensor_mul(out=rq, in0=rq, in1=rqs_sb)   # fold temperature
            sk = vec[:, 66:67]
            nc.vector.tensor_reduce(out=sk, in_=acc_k, axis=AX.X, op=AL.add)
            nc.vector.tensor_scalar_max(out=sk, in0=sk, scalar1=EPS_NRM * EPS_NRM)
            nc.scalar.activation(out=sk, in_=sk, func=AF.Ln)
            rk = vec[:, 67:68]
            nc.scalar.activation(out=rk, in_=sk, func=AF.Exp, scale=-0.5)
            rk128 = vecp.tile([C, 1], bf, tag="rk128")
            nc.vector.tensor_scalar_mul(out=rk128, in0=rk, scalar1=128.0)
            rkt = dramp.tile([C, 1], bf, tag="rkt")
            nc.gpsimd.dma_start(out=rkt, in_=rk128)
            rkrow = vecp.tile([1, C], bf, tag="rkrow")
            nc.gpsimd.dma_start(out=rkrow, in_=rkt.rearrange("c one -> one c"))
            rkb_ps = psg.tile([C, C], f32, tag="rkb")
            nc.tensor.matmul(rkb_ps, ones_sb[0:1, :], rkrow, start=True, stop=True)
            rkb = vecp.tile([C, C], bf, tag="rkb_sb")
            nc.scalar.copy(out=rkb, in_=rkb_ps)

            Lg = vecp.tile([C, 2 * C], f32, tag="Lg")
            L_sb, Pexp = Lg[:, 0:C], Lg[:, C:2 * C]
            # L = (G * rq[c]) * rk[d] * temp, then + head-block mask
            nc.vector.scalar_tensor_tensor(out=L_sb, in0=G, scalar=rq, in1=rkb,
                                           op0=AL.mult, op1=AL.mult)
            nc.vector.tensor_add(out=L_sb, in0=L_sb, in1=maskb_sb)
            mx = vec[:, 68:69]
            nc.vector.tensor_reduce(out=mx, in_=L_sb, axis=AX.X, op=AL.max)
            nmx = vec[:, 69:70]
            nc.vector.tensor_scalar_mul(out=nmx, in0=mx, scalar1=-1.0)
            nc.scalar.activation(out=Pexp, in_=L_sb, func=AF.Exp, bias=nmx)
            den = vec[:, 70:71]
            nc.vector.tensor_reduce(out=den, in_=Pexp, axis=AX.X, op=AL.add)
            rden = vec[:, 71:72]
            nc.vector.reciprocal(out=rden, in_=den)
            Pg = vecp.tile([C, 2 * C], bf, tag="Pg")
            P_sb, Pt_sb = Pg[:, 0:C], Pg[:, C:2 * C]
            nc.vector.tensor_scalar_mul(out=P_sb, in0=Pexp, scalar1=rden)
            nc.sync.dma_start_transpose(out=Pt_sb, in_=P_sb)

            # ---- phase 4: v stream -> attn apply -> proj -> residual --
            qv = qkvp.tile([C, N], bf, tag="qv")
            lhsT = wqkv_sb[:, 2 * C:3 * C]
            for k in range(32):
                q_ps = ps.tile([C, 512], f32, tag="ps")
                nc.tensor.matmul(q_ps, lhsT, xs_sb[:, 512 * k:512 * (k + 1)],
                                 start=True, stop=True)
                dst = qv[:, 512 * k:512 * (k + 1)]
                if k % 2 == 0:
                    nc.scalar.add(out=dst, in_=q_ps, add=bqkv_sb[:, 2:3])
                else:
                    nc.vector.tensor_scalar_add(out=dst, in0=q_ps,
                                                scalar1=bqkv_sb[:, 2:3])
            for j in range(8):
                res = xin.tile([C, 2048], bf, tag="xc")
                nc.gpsimd.dma_start(out=res, in_=x_d[s, :, 2048 * j:2048 * (j + 1)])
                oc = outp.tile([C, 2048], bf, tag="oc")
                for kk in range(4):
                    k = 4 * j + kk
                    ch = dwch.tile([C, 512], bf, tag="ch")
                    ps_dw = dw_pe_taps(ps, qv, wdiag_sb, 2, k)
                    nc.scalar.add(out=ch, in_=ps_dw, add=bdw_sb[:, 2:3])
                    dw_dve_taps(qv, ch, wdve_sb, 2, k)
                    av_ps = ps.tile([C, 512], f32, tag="ps")
                    nc.tensor.matmul(av_ps, Pt_sb, ch, start=True, stop=True)
                    att = attp.tile([C, 512], bf, tag="att")
                    nc.scalar.copy(out=att, in_=av_ps)
                    pj_ps = ps.tile([C, 512], f32, tag="ps")
                    nc.tensor.matmul(pj_ps, wproj_sb, att, start=True, stop=True)
                    pj = prjp.tile([C, 512], bf, tag="pj")
                    nc.scalar.add(out=pj, in_=pj_ps, add=bproj_sb)
                    ssl = slice(512 * kk, 512 * (kk + 1))
                    nc.vector.scalar_tensor_tensor(
                        out=oc[:, ssl], in0=res[:, ssl], scalar=grw_sb, in1=pj,
                        op0=AL.mult, op1=AL.add)
                nc.gpsimd.dma_start(out=out_d[s, :, 2048 * j:2048 * (j + 1)], in_=oc)

    _split_waits()
    return nc


# ---------------------------------------------------------------------------
# Host side: weight prep, sharding, cached PJRT runner
# ---------------------------------------------------------------------------

def _prep_core_inputs(x, ln_w, ln_b, qkv_w, qkv_b, dw_w, dw_b, proj_w, proj_b,
                      temperature, grw):
    import ml_dtypes
    bf = ml_dtypes.bfloat16

    tiles = x.reshape(B, C, 3, HH, 3, WW).transpose(2, 4, 0, 1, 3, 5) \
             .reshape(T, B, C, HH, WW)
    items_x = tiles.reshape(T * B, C, N)
    pad = N_CORES * SLOTS - T * B
    items_x = np.concatenate(
        [items_x, np.zeros((pad, C, N), items_x.dtype)]).astype(bf)
    t_idx = np.concatenate([np.arange(T * B) // B, np.zeros(pad, np.int64)])

    WQKV = np.zeros((T, C, 3 * C), np.float32)
    BQKV = np.zeros((T, C, 3), np.float32)
    WDIAG = np.zeros((T, C, 18 * C), np.float32)
    WDVE = np.zeros((T, C, 9), np.float32)
    BDW = np.zeros((T, C, 3), np.float32)
    WPROJ = np.zeros((T, C, C), np.float32)
    for t in range(T):
        Wp = qkv_w[t] * ln_w[t][None, :]           # [384, 128]
        Wc = Wp - Wp.mean(axis=1, keepdims=True)   # fold LN mean removal
        bp = qkv_b[t] + qkv_w[t] @ ln_b[t]         # [384]
        for g in range(3):
            WQKV[t, :, C * g:C * (g + 1)] = Wc[C * g:C * (g + 1), :].T
            BQKV[t, :, g] = bp[C * g:C * (g + 1)]
            BDW[t, :, g] = dw_b[t, C * g:C * (g + 1)]
            for j, (dr, dc) in enumerate(PE_TAPS):
                d = np.diag(dw_w[t, C * g:C * (g + 1), 0, dr + 1, dc + 1])
                WDIAG[t, :, C * (6 * g + j):C * (6 * g + j + 1)] = d
            for dr in (-1, 0, 1):
                WDVE[t, :, 3 * g + dr + 1] = dw_w[t, C * g:C * (g + 1), 0, dr + 1, 1]
        WPROJ[t] = proj_w[t].T

    RQS = np.repeat(temperature, C // HEADS, axis=1)[:, :, None]  # [T, C, 1]
    GRW = np.broadcast_to(np.asarray(grw)[:, None, None], (T, C, 1))

    mask = np.full((C, C), NEG, np.float32)
    for h in range(HEADS):
        mask[16 * h:16 * (h + 1), 16 * h:16 * (h + 1)] = 0.0
    ones = np.full((C, C), 1.0 / C, np.float32)

    in_maps = []
    for c in range(N_CORES):
        ts = t_idx[c * SLOTS:(c + 1) * SLOTS]
        in_maps.append(dict(
            x=np.ascontiguousarray(items_x[c * SLOTS:(c + 1) * SLOTS]),
            wqkv=WQKV[ts].astype(bf),
            bqkv=np.ascontiguousarray(BQKV[ts]),
            wdiag=WDIAG[ts].astype(bf),
            wdve=np.ascontiguousarray(WDVE[ts]),
            bdw=np.ascontiguousarray(BDW[ts]),
            wproj=WPROJ[ts].astype(bf),
            bproj=np.ascontiguousarray(proj_b[ts][:, :, None]).astype(np.float32),
            rqs=np.ascontiguousarray(RQS[ts]).astype(np.float32),
            grw=np.ascontiguousarray(GRW[ts]).astype(np.float32),
            onesw=ones.astype(bf),
            maskb=mask,
        ))
    return in_maps


def _get_runner():
    if "run" in _cache:
        return _cache["run"]

    import jax
    from jax.sharding import Mesh, PartitionSpec
    try:
        from jax.experimental.shard_map import shard_map
    except ImportError:
        from jax import shard_map
    from concourse import bass2jax, mybir

    nc = _build_bass()
    bass2jax.install_neuronx_cc_hook()

    in_names, out_names, out_avals, zero_outs = [], [], [], []
    for alloc in nc.m.functions[0].allocations:
        if not isinstance(alloc, mybir.MemoryLocationSet):
            continue
        name = alloc.memorylocations[0].name
        if alloc.kind == "ExternalInput":
            in_names.append(name)
        elif alloc.kind == "ExternalOutput":
            out_names.append(name)
            shape = tuple(alloc.tensor_shape)
            dtype = mybir.dt.np(alloc.dtype)
            out_avals.append(jax.core.ShapedArray(shape, dtype))
            zero_outs.append(np.zeros(shape, dtype))
    n_params = len(in_names)
    all_names = in_names + out_names
    donate = tuple(range(n_params, n_params + len(out_names)))

    def _body(*args):
        outs = bass2jax._bass_exec_p.bind(
            *args,
            out_avals=tuple(out_avals),
            in_names=tuple(all_names),
            out_names=tuple(out_names),
            lowering_input_output_aliases=(),
            sim_require_finite=True,
            sim_require_nnan=True,
            nc=nc,
        )
        return tuple(outs)

    devices = jax.devices()[:N_CORES]
    mesh = Mesh(np.asarray(devices), ("core",))
    in_specs = (PartitionSpec("core"),) * (n_params + len(out_names))
    out_specs = (PartitionSpec("core"),) * len(out_names)
    smapped = shard_map(_body, mesh=mesh, in_specs=in_specs,
                        out_specs=out_specs, check_rep=False)
    sharded = jax.jit(smapped, donate_argnums=donate, keep_unused=True)
    sharded_nd = jax.jit(smapped, keep_unused=True)   # repeat-callable variant

    run = dict(fn=sharded, fn_nd=sharded_nd, in_names=in_names,
               out_names=out_names, zero_outs=zero_outs, n_params=n_params)
    _cache["run"] = run
    return run


def _concat_inputs(run, in_maps):
    return [np.concatenate([m[name] for m in in_maps], axis=0)
            for name in run["in_names"]]


def _fresh_zeros(run):
    return [np.zeros((N_CORES * z.shape[0],) + z.shape[1:], z.dtype)
            for z in run["zero_outs"]]


def kernel(x, ln_w, ln_b, qkv_w, qkv_b, dw_w, dw_b, proj_w, proj_b,
           temperature, grw):
    run = _get_runner()
    in_maps = _prep_core_inputs(
        np.asarray(x, np.float32), np.asarray(ln_w), np.asarray(ln_b),
        np.asarray(qkv_w), np.asarray(qkv_b), np.asarray(dw_w),
        np.asarray(dw_b), np.asarray(proj_w), np.asarray(proj_b),
        np.asarray(temperature), np.asarray(grw))
    out_arrs = run["fn"](*_concat_inputs(run, in_maps), *_fresh_zeros(run))
    out = np.asarray(out_arrs[0]).astype(np.float32)    # [40, C, N]
    out = out.reshape(N_CORES * SLOTS, C, HH, WW)[:T * B]
    out = out.reshape(3, 3, B, C, HH, WW).transpose(2, 3, 0, 4, 1, 5) \
             .reshape(B, C, H, W)
    return np.ascontiguousarray(out, dtype=np.float32)
